# revision 69
# baseline (speedup 1.0000x reference)
"""Distributed causal multi-head attention for 8 TRN2 NeuronCores.

Problem: x[2, 2048, 1024], 16 heads x 64 dim, causal softmax attention,
output projection. Sharding: tensor-parallel over (batch, head-group):
core c handles batch c//4 and heads [4*(c%4), 4*(c%4)+4). Each core
computes its 4 heads' attention plus the partial output projection
(sum over its heads); the host sums the 4 partials per batch.

On-device layout strategy (no transposes anywhere on device):
  - host feeds xT = x[b].T               [D=1024, S=2048]
  - wq/wk/wv = W[heads] as [D, 256]      (d-major, head-major columns)
  - wo pair  = W_O rows per head pair    [128, 1024]
  - Q^T/K^T computed as [head-pair 128, S]; V as [p, 65*4] with a ones
    column folded per head so the attention-value matmul also produces
    the softmax denominator row.
  - scores tile per (p-tile, head pair) is one 2-bank PSUM tile laid
    [h0 | h1] with the causally-dead c0 columns squeezed out so a single
    contiguous ACT exp covers both heads with no garbage columns (ACT
    paces the attention pipeline; every element counts).
  - causal handling: fully-masked 128-col blocks are skipped in
    scores/exp/AV; the true-diagonal 128x128 block gets a multiplicative
    tril on the probabilities after exp (keeps DVE off the ACT path).
  - z^T accumulated in PSUM [65, 512] per head (row 64 = denominator l).
  - normalization (deferred one q-chunk, emitted at step 7 of the next
    chunk so the PE never waits on it): l rows gathered to partitions
    32h, one [128,512] reciprocal on DVE, bf16 cast on gpsimd, then a
    one-hot [128,128] indicator matmul per head pair broadcasts r across
    partitions into a recycled sc-ring PSUM tile; DVE muls produce the
    normalized pair tile zup[hp] [128, S] consumed by the O-projection.
  - O-projection: out[q,1024] accumulates TWO K=128 matmuls per output
    tile (head pairs stacked on the contraction axis) - full PE array,
    half the instruction count of per-head K=64 matmuls.

Matmul compute dtype: bfloat16 (full-rate on TRN2; rel err ~6e-3 vs the
fp32 reference), fp32 accumulation in PSUM. The per-q 1/l factors are
bf16 (adds <0.4% rms; the 2e-2 gate has plenty of margin).

Schedule notes:
  - Score matmuls for the two heads of a pair sit at PE row groups 0/64
    (tile_position) and execute CONCURRENTLY on the array - confirmed
    from the perfetto trace (pairs overlap ~90%).
  - AV matmuls run LAG=3 pair-steps behind scores to hide exp latency.
  - The attention phase's exp work (~90us at ~115G elem/s on ACT) is
    spread into every ACT-idle window: the first two q-chunks' scores
    are dripped one pair per projection di-group (exps run under the
    projection matmuls), the third chunk's hp0 scores drip through the
    first two chunks' (AV-dominated) attention, and the earlier chunks'
    O-projection tiles are absorbed into the exp-bound fourth chunk's
    stream via recycled sc-ring PSUM tiles. Held P tiles wait in a deep
    ring (bufs=36) until their AV matmuls consume them; only the last
    chunk's reciprocal chain + 4 output tiles remain as the tail.
"""

import sys

if "/opt/trn_rl_repo" not in sys.path:
    sys.path.insert(0, "/opt/trn_rl_repo")

import numpy as np

import concourse.bass as bass
import concourse.mybir as mybir
import concourse.tile as tile
from concourse.bass_utils import run_bass_kernel_spmd

B = 2
S = 2048
D = 1024
NH = 16
DH = 64
N_CORES = 8
HPC = 4          # heads per core
HL = HPC * DH    # 256 local head dims
QC = 512         # q-chunk width
NQC = S // QC

F32 = mybir.dt.float32
BF16 = mybir.dt.bfloat16
EXP = mybir.ActivationFunctionType.Exp


def _split_multiwait(nc, max_waits=1):
    """Walrus (CoreV3) rejects instructions carrying more than one sync
    wait; split extras into single-wait nops inserted before, same engine."""
    for f in nc.m.functions:
        for blk in f.blocks:
            insts = blk.instructions
            idx = 0
            while idx < len(insts):
                inst = insts[idx]
                si = getattr(inst, "sync_info", None)
                waits = list(si.on_wait) if si is not None else []
                if len(waits) > max_waits:
                    extra, keep = waits[:-max_waits], waits[-max_waits:]
                    si.on_wait = keep
                    for j, w in enumerate(extra):
                        nop = mybir.InstNoOp(
                            name=f"{inst.name}_sw{j}",
                            engine=inst.engine,
                            sync_info=mybir.SyncInfo(on_wait=[w], on_update=[]),
                            bass_nofuse=True,
                        )
                        insts.insert(idx, nop)
                        idx += 1
                idx += 1


def build_nc():
    nc = bass.Bass("TRN2", target_bir_lowering=False, debug=False, num_devices=N_CORES)

    xT_d = nc.declare_dram_parameter("xT", [D, S], BF16, isOutput=False)
    wq_d = nc.declare_dram_parameter("wq", [D, HL], BF16, isOutput=False)
    wk_d = nc.declare_dram_parameter("wk", [D, HL], BF16, isOutput=False)
    wv_d = nc.declare_dram_parameter("wv", [D, HL], BF16, isOutput=False)
    wo_d = nc.declare_dram_parameter("wo", [HL, D], BF16, isOutput=False)
    mask_d = nc.declare_dram_parameter("mask", [128, 128], BF16, isOutput=False)
    out_d = nc.declare_dram_parameter("out", [S, D], BF16, isOutput=True)

    with tile.TileContext(nc) as tc:
        with (
            tc.tile_pool(name="live_sb", bufs=1) as live_sb,
            tc.tile_pool(name="att_sb", bufs=1) as att_sb,
        ):
            # Tensors that live through the whole kernel.
            QT = [live_sb.tile([128, S], BF16, tag=f"QT{hc}", name=f"QT{hc}") for hc in range(2)]
            KT = [live_sb.tile([128, S], BF16, tag=f"KT{hc}", name=f"KT{hc}") for hc in range(2)]
            # V with a ones column per head: 16 p-chunks x [V0|1|V1|1|V2|1|V3|1]
            V_sb = live_sb.tile([128, 16 * (HPC * 65)], BF16, tag="V", name="V")
            wop = [live_sb.tile([128, D], BF16, tag=f"wop{hp}", name=f"wop{hp}") for hp in range(2)]
            mask_t = live_sb.tile([128, 128], BF16, tag="mask", name="mask")

            # Indicator weights for the r broadcast matmul (bf16: full PE
            # rate; the f32->bf16 cast of 1/l runs on the idle gpsimd well
            # off the critical path). Head h's 1/l row lives at partition
            # 32h (engine ops need 32-aligned bases). ind_hp is a one-hot
            # [128, 128] selecting partition 64hp -> output rows 0-63 and
            # 64hp+32 -> rows 64-127.
            indf = live_sb.tile([128, 128], F32, tag="indf", name="indf")
            ind = [live_sb.tile([128, 128], BF16, tag=f"ind{hp}", name=f"ind{hp}")
                   for hp in range(2)]
            for hp in range(2):
                nc.vector.memset(indf[:, :], 0.0)
                nc.vector.memset(indf[64 * hp:64 * hp + 1, 0:64], 1.0)
                nc.vector.memset(indf[64 * hp + 32:64 * hp + 33, 64:128], 1.0)
                nc.vector.tensor_copy(ind[hp][:, :], indf[:, :])

            # ones columns of V (col 64 of each head block), one strided memset
            ones_ap = V_sb[:, :].rearrange("p (a c) -> p a c", c=65)[:, :, 64:65]
            nc.vector.memset(ones_ap, 1.0)

            # The score-tile PSUM ring is open through phases 1+2: the
            # first two q-chunks' hp0 scores (and qc1's hp1) are emitted
            # during the projection phase so their exps run on the
            # otherwise-idle ACT engine under the projection matmuls.
            sc_pool_cm = tc.tile_pool(name="sc_ps", bufs=2, space="PSUM")
            sc_ps = sc_pool_cm.__enter__()
            PS_CACHE = {}

            def emit_scores_g(qc_, pt, hp):
                q0_ = qc_ * QC
                jj = pt - q0_ // 128  # >=0 means diagonal region
                c0 = max(0, jj) * 128
                # pair layout [h0 | h1] with dead columns squeezed:
                # h0 -> cols [c0:512], h1 -> cols [512:1024-c0], so one
                # contiguous exp covers both heads, no garbage.
                scp = sc_ps.tile([128, 1024], F32, tag="sc", name="sc")
                for i in range(2):
                    h = 2 * hp + i
                    hc2, ho = h // 2, (h % 2) * 64
                    dst = (scp[:, c0:512] if i == 0 else scp[:, 512:1024 - c0])
                    nc.tensor.matmul(
                        dst,
                        KT[hc2][ho:ho + 64, pt * 128:pt * 128 + 128],
                        QT[hc2][ho:ho + 64, q0_ + c0:q0_ + QC],
                        start=True,
                        stop=True,
                        tile_position=(ho, 0),
                    )
                Pp = att_sb.tile([128, 1024], BF16, tag="P", name="P", bufs=40)
                nc.scalar.activation(
                    Pp[:, c0:1024 - c0], scp[:, c0:1024 - c0], EXP, scale=0.125
                )
                if jj >= 0:
                    # causal tril applied multiplicatively post-exp; all-SBUF
                    # bf16 work on the otherwise idle gpsimd (DVE saturates
                    # at chunk boundaries; LAG gives gpsimd plenty of slack)
                    blk0 = slice(jj * 128, (jj + 1) * 128)
                    nc.gpsimd.tensor_mul(Pp[:, blk0], Pp[:, blk0], mask_t[:, :])
                    nc.gpsimd.tensor_mul(Pp[:, 512:640], Pp[:, 512:640], mask_t[:, :])
                PS_CACHE[(qc_, pt, hp)] = Pp

            # ---- Phase 1: projections (xT and w tiles scoped here) ----
            with (
                tc.tile_pool(name="xw_sb", bufs=1) as xw_sb,
                tc.tile_pool(name="proj_ps", bufs=4, space="PSUM") as proj_ps,
            ):
                # DMA issue order matters: the projection's first matmul
                # needs wq + xT chunk 0, so those go first; wk before the
                # x tail so the K^T loop never waits on it.
                w_sb = {}
                w_tiles = {}
                for name in ("wq", "wk", "wv"):
                    w_tiles[name] = xw_sb.tile(
                        [128, 8 * HL], BF16, tag=f"{name}b", name=f"{name}b"
                    )

                def _w_dma(name, dram):
                    t = w_tiles[name]
                    src = dram.ap().rearrange("(di p) h -> di p h", p=128).transpose((1, 0, 2))
                    dst = t[:, :].rearrange("p (di h) -> p di h", di=8)
                    nc.sync.dma_start(out=dst, in_=src)
                    w_sb[name] = t

                xT_t = [
                    xw_sb.tile([128, S], BF16, tag=f"x{di}", name=f"x{di}")
                    for di in range(8)
                ]

                def _x_dma(di, eng=nc.sync):
                    eng.dma_start(
                        out=xT_t[di][:, :], in_=xT_d[di * 128:(di + 1) * 128, :]
                    )

                _w_dma("wq", wq_d)
                _x_dma(0)
                _x_dma(1)
                _w_dma("wk", wk_d)
                for di in range(2, 8):
                    _x_dma(di)
                _w_dma("wv", wv_d)
                nc.sync.dma_start(out=mask_t[:, :], in_=mask_d[:, :])
                for hp in range(2):
                    nc.sync.dma_start(
                        out=wop[hp][:, :], in_=wo_d[hp * 128:(hp + 1) * 128, :]
                    )

                def w_t_slice(name, di, lo, hi):
                    return w_sb[name][:, di * HL + lo:di * HL + hi]

                # Q^T, K^T: [head-pair 128, S]. di outer / qt inner so the
                # stationary weight chunk is reused across 4 matmuls.
                # PSUM drains split vector/scalar (ACT is idle in phase 1).
                # Group order hc-major: after (wk, hc0) both hp0 tensors
                # exist, so the first chunks' hp0 scores start right away.
                # Early-score drip: one score pair emitted between the
                # projection's di-groups, so the exps stream on the idle
                # ACT engine without the sc-ring (bufs=2) ever throttling
                # the PE to ACT pace.
                early_q = []

                def drip():
                    if early_q:
                        emit_scores_g(*early_q.pop(0))

                ndr = 0
                for wname, hc in (("wq", 0), ("wk", 0), ("wq", 1), ("wk", 1)):
                    dst = QT if wname == "wq" else KT
                    pss = [
                        proj_ps.tile([128, 512], F32, tag="pp", name="pp")
                        for _ in range(4)
                    ]
                    for di in range(8):
                        for qt in range(4):
                            nc.tensor.matmul(
                                pss[qt][:, :],
                                w_t_slice(wname, di, hc * 128, (hc + 1) * 128),
                                xT_t[di][:, qt * 512:(qt + 1) * 512],
                                start=(di == 0),
                                stop=(di == 7),
                            )
                        if di % 2 == 1:
                            drip()
                    for qt in range(4):
                        eng = nc.vector if ndr % 2 == 0 else nc.scalar
                        if eng is nc.scalar:
                            eng.copy(
                                dst[hc][:, qt * 512:(qt + 1) * 512], pss[qt][:, :]
                            )
                        else:
                            eng.tensor_copy(
                                dst[hc][:, qt * 512:(qt + 1) * 512], pss[qt][:, :]
                            )
                        ndr += 1
                    if (wname, hc) == ("wk", 0):
                        # qc0+qc1 hp0 scores: their exps run under the
                        # remaining QK projection matmuls
                        for qc_e in (0, 1):
                            for pt in range(qc_e * 4 + 4):
                                early_q.append((qc_e, pt, 0))
                    if (wname, hc) == ("wk", 1):
                        # qc1 hp1 scores: exps run under the V projection
                        for pt in range(8):
                            early_q.append((1, pt, 1))

                # V: [p, h] per p-chunk; one strided drain per chunk writes
                # around the pre-set ones columns.
                for pc in range(16):
                    ps = proj_ps.tile([128, 512], F32, tag="pp", name="pp")
                    for di in range(8):
                        nc.tensor.matmul(
                            ps[:, :HL],
                            xT_t[di][:, pc * 128:(pc + 1) * 128],
                            w_t_slice("wv", di, 0, HL),
                            start=(di == 0),
                            stop=(di == 7),
                        )
                    base = pc * (HPC * 65)
                    vdst = V_sb[:, base:base + HPC * 65].rearrange(
                        "p (h c) -> p h c", c=65
                    )[:, :, 0:64]
                    vsrc = ps[:, :HL].rearrange("p (h c) -> p h c", c=64)
                    # vector only: the ACT queue is full of early exps and
                    # the first AVs need V as soon as the projection ends
                    nc.vector.tensor_copy(vdst, vsrc)
                    drip()
                while early_q:
                    drip()

            # ---- Phase 2: attention (normalization deferred 1 q-chunk) ----
            # zu: unnormalized z^T per head [64, S]; lall row h = denominator
            # of head h; rqb = bf16(1/l). zup: normalized head-pair tiles
            # [128, S] feeding the K=128 O-projection.
            zu = [att_sb.tile([64, S], BF16, tag=f"zu{h}", name=f"zu{h}")
                  for h in range(HPC)]
            zup = [att_sb.tile([128, S], BF16, tag=f"zup{hp}", name=f"zup{hp}")
                   for hp in range(2)]
            # head h's denominator / reciprocal parked at partition 32h
            lall = att_sb.tile([128, S], F32, tag="lall", name="lall")
            rall = att_sb.tile([128, S], F32, tag="rall", name="rall")
            rqb = att_sb.tile([128, S], BF16, tag="rqb", name="rqb")
            nc.vector.memset(lall[:, :], 1.0)
            with tc.tile_pool(name="z_ps", bufs=4, space="PSUM") as z_ps:
                pending_norm = None
                pending_oproj = []

                def make_oproj(q0, qs):
                    # one output tile [128 q, 1024]: 4 K=128 pair matmuls in
                    # a recycled sc-ring tile, engine drain to bf16, DMA out
                    def _oproj(drain_eng=None):
                        pso = sc_ps.tile([128, 1024], F32, tag="sc", name="sc")
                        for dm in range(2):
                            for hp in range(2):
                                nc.tensor.matmul(
                                    pso[:, dm * 512:(dm + 1) * 512],
                                    zup[hp][:, q0 + qs * 128:q0 + (qs + 1) * 128],
                                    wop[hp][:, dm * 512:(dm + 1) * 512],
                                    start=(hp == 0),
                                    stop=(hp == 1),
                                )
                        ot = att_sb.tile([128, D], BF16, tag="ot", name="ot", bufs=4)
                        if drain_eng is nc.scalar:
                            nc.scalar.copy(ot[:, :], pso[:, :])
                        else:
                            nc.vector.tensor_copy(ot[:, :], pso[:, :])
                        nc.sync.dma_start(
                            out=out_d[q0 + qs * 128:q0 + (qs + 1) * 128, :],
                            in_=ot[:, :],
                        )
                    return _oproj

                def make_norm(q0):
                    # rb broadcast into a recycled sc-ring tile (both pairs
                    # side by side; f32 matmul straight from the f32
                    # reciprocal) + normalized pair tiles on vector. Using
                    # the sc ring instead of the z ring means this can be
                    # emitted late enough that the PE never waits on the
                    # DVE reciprocal chain.
                    def _norm():
                        rbt = sc_ps.tile([128, 1024], F32, tag="sc", name="sc")
                        for hp in range(2):
                            nc.tensor.matmul(
                                rbt[:, hp * 512:(hp + 1) * 512],
                                ind[hp][:, :], rqb[:, q0:q0 + QC],
                                start=True, stop=True,
                            )
                        for hp in range(2):
                            for i in range(2):
                                nc.vector.tensor_mul(
                                    zup[hp][64 * i:64 * i + 64, q0:q0 + QC],
                                    zu[2 * hp + i][:, q0:q0 + QC],
                                    rbt[64 * i:64 * i + 64, hp * 512:(hp + 1) * 512],
                                )
                    return _norm

                for qc in range(NQC):
                    q0 = qc * QC
                    npt = q0 // 128 + 4
                    zt = []  # allocated lazily at the first AV step
                    Ps = {}

                    def emit_scores(pt, hp):
                        if (qc, pt, hp) in PS_CACHE:
                            Ps[(pt, hp)] = PS_CACHE.pop((qc, pt, hp))
                            return
                        emit_scores_g(qc, pt, hp)
                        Ps[(pt, hp)] = PS_CACHE.pop((qc, pt, hp))

                    def emit_av(apt, hp):
                        ac0 = max(0, apt - q0 // 128) * 128
                        Pp = Ps.pop((apt, hp))
                        if not zt:
                            zt.extend(
                                z_ps.tile([128, 512], F32, tag="z", name="z")
                                for _ in range(HPC)
                            )
                        for i in range(2):
                            h = 2 * hp + i
                            src = (Pp[:, ac0:512] if i == 0
                                   else Pp[:, 512:1024 - ac0])
                            nc.tensor.matmul(
                                zt[h][0:65, ac0:],
                                V_sb[:, apt * (HPC * 65) + h * 65: apt * (HPC * 65) + (h + 1) * 65],
                                src,
                                start=(apt == 0),
                                stop=(apt == npt - 1),
                            )

                    # pair-granular software pipeline, 3-pair lag so the exp
                    # latency stays off the PE critical path. The previous
                    # chunk's deferred normalization slots in at step 6 (the
                    # reciprocal chain is done by then - no PE stall). While
                    # the first two chunks run on pre-computed P tiles, the
                    # third chunk's hp0 scores drip in to keep ACT fed.
                    steps = [(pt, hp) for pt in range(npt) for hp in range(2)]
                    LAG = 4
                    n_iter = len(steps) + LAG
                    sched = {}
                    if pending_norm is not None:
                        sched[min(7, n_iter - 1)] = [pending_norm]
                        pending_norm = None
                    if qc == 0:
                        for k, pt in enumerate(range(0, 4)):
                            sched.setdefault(3 + 2 * k, []).append(
                                (lambda pt=pt: emit_scores_g(2, pt, 0))
                            )
                    elif qc == 1:
                        for k, pt in enumerate(range(4, 12)):
                            sched.setdefault(3 + 2 * k, []).append(
                                (lambda pt=pt: emit_scores_g(2, pt, 0))
                            )
                    elif qc == 2:
                        # a slice of qc3's hp0 scores: qc2's ACT has ~15%
                        # headroom, every exp moved off qc3 shortens its
                        # exp-bound span
                        for k, pt in enumerate(range(0, 4)):
                            sched.setdefault(5 + 3 * k, []).append(
                                (lambda pt=pt: emit_scores_g(3, pt, 0))
                            )
                    elif qc == 3:
                        # qc3 is exp-bound with PE and DVE slack: absorb the
                        # earlier chunks' O-projection tiles here (deps long
                        # settled; qc2's zup lands by ~step 9)
                        for k, fn in enumerate(pending_oproj):
                            sched.setdefault(min(8 + 2 * k, n_iter - 1), []).append(fn)
                        pending_oproj = []
                    for n in range(n_iter):
                        if n < len(steps):
                            emit_scores(*steps[n])
                        for fn in sched.get(n, ()):
                            fn()
                        if n >= LAG:
                            emit_av(*steps[n - LAG])
                            # heads 0/1 stop one step before 2/3: start the
                            # reciprocal chain + their z drains early
                            if steps[n - LAG] == (npt - 1, 0):
                                nc.scalar.copy(
                                    lall[0:1, q0:q0 + QC], zt[0][64:65, 0:512]
                                )
                                nc.vector.tensor_copy(
                                    lall[32:33, q0:q0 + QC], zt[1][64:65, 0:512]
                                )
                                nc.scalar.copy(
                                    zu[0][:, q0:q0 + QC], zt[0][0:64, 0:512]
                                )
                                nc.vector.tensor_copy(
                                    zu[1][:, q0:q0 + QC], zt[1][0:64, 0:512]
                                )

                    # ---- q-chunk tail: finish l gather, reciprocal, drain z
                    # (split scalar/vector so the reciprocal starts sooner;
                    # single-input PSUM->SBUF copies have no same-base rule)
                    nc.scalar.copy(lall[64:65, q0:q0 + QC], zt[2][64:65, 0:512])
                    nc.vector.tensor_copy(
                        lall[96:97, q0:q0 + QC], zt[3][64:65, 0:512]
                    )
                    nc.vector.reciprocal(rall[:, q0:q0 + QC], lall[:, q0:q0 + QC])
                    nc.gpsimd.tensor_copy(rqb[:, q0:q0 + QC], rall[:, q0:q0 + QC])
                    nc.scalar.copy(zu[2][:, q0:q0 + QC], zt[2][0:64, 0:512])
                    nc.vector.tensor_copy(zu[3][:, q0:q0 + QC], zt[3][0:64, 0:512])

                    pending_norm = make_norm(q0)
                    pending_oproj = pending_oproj + [
                        make_oproj(q0, qs) for qs in range(4)
                    ]

                # tail: last q-chunk's normalization + its 4 output tiles
                # (all earlier chunks' output tiles were absorbed into the
                # qc3 attention stream; ACT is idle here so half the drains
                # go to scalar)
                pending_norm()
                pending_norm = None
                for k, fn in enumerate(pending_oproj):
                    fn(drain_eng=nc.scalar if k % 2 == 0 else nc.vector)
                pending_oproj = []

            sc_pool_cm.__exit__(None, None, None)

    _split_multiwait(nc)
    return nc


def _prep_in_maps(x, W_K, W_Q, W_V, W_O):
    x = np.asarray(x, dtype=np.float32)
    W_K = np.asarray(W_K, dtype=np.float32)
    W_Q = np.asarray(W_Q, dtype=np.float32)
    W_V = np.asarray(W_V, dtype=np.float32)
    W_O = np.asarray(W_O, dtype=np.float32)

    import ml_dtypes
    bf16 = ml_dtypes.bfloat16
    pp, qq = np.meshgrid(np.arange(128), np.arange(128), indexing="ij")
    mask = np.where(qq >= pp, 1.0, 0.0).astype(bf16)

    in_maps = []
    for c in range(N_CORES):
        b, g = c // 4, c % 4
        hs = slice(HPC * g, HPC * g + HPC)
        xT = np.ascontiguousarray(x[b].T).astype(bf16)
        wq = np.ascontiguousarray(W_Q[hs].transpose(2, 0, 1).reshape(D, HL)).astype(bf16)
        wk = np.ascontiguousarray(W_K[hs].transpose(2, 0, 1).reshape(D, HL)).astype(bf16)
        wv = np.ascontiguousarray(W_V[hs].transpose(2, 0, 1).reshape(D, HL)).astype(bf16)
        wo = np.ascontiguousarray(W_O[:, HL * g:HL * g + HL].T).astype(bf16)
        in_maps.append(
            {"xT": xT, "wq": wq, "wk": wk, "wv": wv, "wo": wo, "mask": mask}
        )
    return in_maps


_NC_CACHE = None


def _get_nc():
    global _NC_CACHE
    if _NC_CACHE is None:
        _NC_CACHE = build_nc()
    return _NC_CACHE


def _run(x, W_K, W_Q, W_V, W_O, trace=False):
    nc = _get_nc()
    in_maps = _prep_in_maps(x, W_K, W_Q, W_V, W_O)
    res = run_bass_kernel_spmd(
        nc, in_maps, core_ids=list(range(N_CORES)), trace=trace
    )
    partials = np.stack(
        [np.asarray(res.results[c]["out"]).astype(np.float32) for c in range(N_CORES)]
    )
    out = np.empty((B, S, D), dtype=np.float32)
    out[0] = partials[0:4].sum(axis=0)
    out[1] = partials[4:8].sum(axis=0)
    return out, res


def kernel(x, W_K, W_Q, W_V, W_O):
    out, _ = _run(x, W_K, W_Q, W_V, W_O, trace=False)
    return out


def run_traced(x, W_K, W_Q, W_V, W_O):
    """For test.py: returns (out, BassKernelResults with exec_time_ns)."""
    import types

    if "antenv.axon_hooks" not in sys.modules:
        try:
            from trn_agent_boot.trn_boot import _ntff_profile_via_ctypes

            hook = _ntff_profile_via_ctypes("/opt/axon/libaxon_pjrt.so")
            mod = types.ModuleType("antenv.axon_hooks")
            mod.get_axon_ntff_profile_hook = lambda: hook
            mod.set_axon_ntff_profile_hook = lambda h: None
            sys.modules["antenv.axon_hooks"] = mod
        except Exception:
            pass
    return _run(x, W_K, W_Q, W_V, W_O, trace=True)


# revision 73
# speedup vs baseline: 1.0195x; 1.0195x over previous
"""Distributed causal multi-head attention for 8 TRN2 NeuronCores.

Problem: x[2, 2048, 1024], 16 heads x 64 dim, causal softmax attention,
output projection. Sharding: tensor-parallel over (batch, head-group):
core c handles batch c//4 and heads [4*(c%4), 4*(c%4)+4). Each core
computes its 4 heads' attention plus the partial output projection
(sum over its heads); the host sums the 4 partials per batch.

On-device layout strategy (no transposes anywhere on device):
  - host feeds xT = x[b].T               [D=1024, S=2048]
  - wq/wk/wv = W[heads] as [D, 256]      (d-major, head-major columns)
  - wo pair  = W_O rows per head pair    [128, 1024]
  - Q^T/K^T computed as [head-pair 128, S]; V as [p, 65*4] with a ones
    column folded per head so the attention-value matmul also produces
    the softmax denominator row.
  - scores tile per (p-tile, head pair) is one 2-bank PSUM tile laid
    [h0 | h1] with the causally-dead c0 columns squeezed out so a single
    contiguous ACT exp covers both heads with no garbage columns (ACT
    paces the attention pipeline; every element counts).
  - causal handling: fully-masked 128-col blocks are skipped in
    scores/exp/AV; the true-diagonal 128x128 block gets a multiplicative
    tril on the probabilities after exp (keeps DVE off the ACT path).
  - z^T accumulated in PSUM [65, 512] per head (row 64 = denominator l).
  - normalization (deferred one q-chunk, emitted at step 7 of the next
    chunk so the PE never waits on it): l rows gathered to partitions
    32h, one [128,512] reciprocal on DVE, bf16 cast on gpsimd, then a
    one-hot [128,128] indicator matmul per head pair broadcasts r across
    partitions into a recycled sc-ring PSUM tile; DVE muls produce the
    normalized pair tile zup[hp] [128, S] consumed by the O-projection.
  - O-projection: out[q,1024] accumulates TWO K=128 matmuls per output
    tile (head pairs stacked on the contraction axis) - full PE array,
    half the instruction count of per-head K=64 matmuls.

Matmul compute dtype: bfloat16 (full-rate on TRN2; rel err ~6e-3 vs the
fp32 reference), fp32 accumulation in PSUM. The per-q 1/l factors are
bf16 (adds <0.4% rms; the 2e-2 gate has plenty of margin).

Schedule notes:
  - Score matmuls for the two heads of a pair sit at PE row groups 0/64
    (tile_position) and execute CONCURRENTLY on the array - confirmed
    from the perfetto trace (pairs overlap ~90%).
  - AV matmuls run LAG=4 pair-steps behind scores to hide exp latency.
  - The attention phase's exp work (~90us at ~115G elem/s on ACT) is
    spread into every ACT-idle window: the first two q-chunks' scores
    are dripped one pair per projection di-group (exps run under the
    projection matmuls), the third chunk's hp0 scores drip through the
    first two chunks' (AV-dominated) attention, and the earlier chunks'
    O-projection tiles are absorbed into the exp-bound fourth chunk's
    stream via recycled sc-ring PSUM tiles. Held P tiles wait in a deep
    ring (bufs=36) until their AV matmuls consume them; only the last
    chunk's reciprocal chain + 4 output tiles remain as the tail.
"""

import sys

if "/opt/trn_rl_repo" not in sys.path:
    sys.path.insert(0, "/opt/trn_rl_repo")

import numpy as np

import concourse.bass as bass
import concourse.mybir as mybir
import concourse.tile as tile
from concourse.bass_utils import run_bass_kernel_spmd

B = 2
S = 2048
D = 1024
NH = 16
DH = 64
N_CORES = 8
HPC = 4          # heads per core
HL = HPC * DH    # 256 local head dims
QC = 512         # q-chunk width
NQC = S // QC

F32 = mybir.dt.float32
BF16 = mybir.dt.bfloat16
EXP = mybir.ActivationFunctionType.Exp


def _split_multiwait(nc, max_waits=1):
    """Walrus (CoreV3) rejects instructions carrying more than one sync
    wait; split extras into single-wait nops inserted before, same engine."""
    for f in nc.m.functions:
        for blk in f.blocks:
            insts = blk.instructions
            idx = 0
            while idx < len(insts):
                inst = insts[idx]
                si = getattr(inst, "sync_info", None)
                waits = list(si.on_wait) if si is not None else []
                if len(waits) > max_waits:
                    extra, keep = waits[:-max_waits], waits[-max_waits:]
                    si.on_wait = keep
                    for j, w in enumerate(extra):
                        nop = mybir.InstNoOp(
                            name=f"{inst.name}_sw{j}",
                            engine=inst.engine,
                            sync_info=mybir.SyncInfo(on_wait=[w], on_update=[]),
                            bass_nofuse=True,
                        )
                        insts.insert(idx, nop)
                        idx += 1
                idx += 1


def build_nc():
    nc = bass.Bass("TRN2", target_bir_lowering=False, debug=False, num_devices=N_CORES)

    xT_d = nc.declare_dram_parameter("xT", [D, S], BF16, isOutput=False)
    wq_d = nc.declare_dram_parameter("wq", [D, HL], BF16, isOutput=False)
    wk_d = nc.declare_dram_parameter("wk", [D, HL], BF16, isOutput=False)
    wv_d = nc.declare_dram_parameter("wv", [D, HL], BF16, isOutput=False)
    wo_d = nc.declare_dram_parameter("wo", [HL, D], BF16, isOutput=False)
    mask_d = nc.declare_dram_parameter("mask", [128, 128], BF16, isOutput=False)
    out_d = nc.declare_dram_parameter("out", [S, D], BF16, isOutput=True)

    with tile.TileContext(nc) as tc:
        with (
            tc.tile_pool(name="live_sb", bufs=1) as live_sb,
            tc.tile_pool(name="att_sb", bufs=1) as att_sb,
        ):
            # Tensors that live through the whole kernel.
            QT = [live_sb.tile([128, S], BF16, tag=f"QT{hc}", name=f"QT{hc}") for hc in range(2)]
            KT = [live_sb.tile([128, S], BF16, tag=f"KT{hc}", name=f"KT{hc}") for hc in range(2)]
            # V with a ones column per head: 16 p-chunks x [V0|1|V1|1|V2|1|V3|1]
            V_sb = live_sb.tile([128, 16 * (HPC * 65)], BF16, tag="V", name="V")
            wop = [live_sb.tile([128, D], BF16, tag=f"wop{hp}", name=f"wop{hp}") for hp in range(2)]
            mask_t = live_sb.tile([128, 128], BF16, tag="mask", name="mask")

            # Indicator weights for the r broadcast matmul (bf16: full PE
            # rate; the f32->bf16 cast of 1/l runs on the idle gpsimd well
            # off the critical path). Head h's 1/l row lives at partition
            # 32h (engine ops need 32-aligned bases). ind_hp is a one-hot
            # [128, 128] selecting partition 64hp -> output rows 0-63 and
            # 64hp+32 -> rows 64-127.
            indf = live_sb.tile([128, 128], F32, tag="indf", name="indf")
            ind = [live_sb.tile([128, 128], BF16, tag=f"ind{hp}", name=f"ind{hp}")
                   for hp in range(2)]
            for hp in range(2):
                nc.vector.memset(indf[:, :], 0.0)
                nc.vector.memset(indf[64 * hp:64 * hp + 1, 0:64], 1.0)
                nc.vector.memset(indf[64 * hp + 32:64 * hp + 33, 64:128], 1.0)
                nc.vector.tensor_copy(ind[hp][:, :], indf[:, :])

            # ones columns of V (col 64 of each head block), one strided memset
            ones_ap = V_sb[:, :].rearrange("p (a c) -> p a c", c=65)[:, :, 64:65]
            nc.vector.memset(ones_ap, 1.0)

            # The score-tile PSUM ring is open through phases 1+2: the
            # first two q-chunks' hp0 scores (and qc1's hp1) are emitted
            # during the projection phase so their exps run on the
            # otherwise-idle ACT engine under the projection matmuls.
            sc_pool_cm = tc.tile_pool(name="sc_ps", bufs=2, space="PSUM")
            sc_ps = sc_pool_cm.__enter__()
            PS_CACHE = {}

            def emit_scores_g(qc_, pt, hp):
                q0_ = qc_ * QC
                jj = pt - q0_ // 128  # >=0 means diagonal region
                c0 = max(0, jj) * 128
                # pair layout [h0 | h1] with dead columns squeezed:
                # h0 -> cols [c0:512], h1 -> cols [512:1024-c0], so one
                # contiguous exp covers both heads, no garbage.
                scp = sc_ps.tile([128, 1024], F32, tag="sc", name="sc")
                for i in range(2):
                    h = 2 * hp + i
                    hc2, ho = h // 2, (h % 2) * 64
                    dst = (scp[:, c0:512] if i == 0 else scp[:, 512:1024 - c0])
                    nc.tensor.matmul(
                        dst,
                        KT[hc2][ho:ho + 64, pt * 128:pt * 128 + 128],
                        QT[hc2][ho:ho + 64, q0_ + c0:q0_ + QC],
                        start=True,
                        stop=True,
                        tile_position=(ho, 0),
                    )
                Pp = att_sb.tile([128, 1024], BF16, tag="P", name="P", bufs=36)
                nc.scalar.activation(
                    Pp[:, c0:1024 - c0], scp[:, c0:1024 - c0], EXP, scale=0.125
                )
                if jj >= 0:
                    # causal tril applied multiplicatively post-exp; all-SBUF
                    # bf16 work on the otherwise idle gpsimd (DVE saturates
                    # at chunk boundaries; LAG gives gpsimd plenty of slack)
                    blk0 = slice(jj * 128, (jj + 1) * 128)
                    nc.gpsimd.tensor_mul(Pp[:, blk0], Pp[:, blk0], mask_t[:, :])
                    nc.gpsimd.tensor_mul(Pp[:, 512:640], Pp[:, 512:640], mask_t[:, :])
                PS_CACHE[(qc_, pt, hp)] = Pp

            # ---- Phase 1: projections (xT and w tiles scoped here) ----
            with (
                tc.tile_pool(name="xw_sb", bufs=1) as xw_sb,
                tc.tile_pool(name="proj_ps", bufs=4, space="PSUM") as proj_ps,
            ):
                # DMA issue order matters: the projection's first matmul
                # needs wq + xT chunk 0, so those go first; wk before the
                # x tail so the K^T loop never waits on it.
                w_sb = {}
                w_tiles = {}
                for name in ("wq", "wk", "wv"):
                    w_tiles[name] = xw_sb.tile(
                        [128, 8 * HL], BF16, tag=f"{name}b", name=f"{name}b"
                    )

                def _w_dma(name, dram):
                    t = w_tiles[name]
                    src = dram.ap().rearrange("(di p) h -> di p h", p=128).transpose((1, 0, 2))
                    dst = t[:, :].rearrange("p (di h) -> p di h", di=8)
                    nc.sync.dma_start(out=dst, in_=src)
                    w_sb[name] = t

                xT_t = [
                    xw_sb.tile([128, S], BF16, tag=f"x{di}", name=f"x{di}")
                    for di in range(8)
                ]

                def _x_dma(di, eng=nc.sync):
                    eng.dma_start(
                        out=xT_t[di][:, :], in_=xT_d[di * 128:(di + 1) * 128, :]
                    )

                _w_dma("wq", wq_d)
                _x_dma(0)
                _x_dma(1)
                _w_dma("wk", wk_d)
                for di in range(2, 8):
                    _x_dma(di)
                _w_dma("wv", wv_d)
                nc.sync.dma_start(out=mask_t[:, :], in_=mask_d[:, :])
                for hp in range(2):
                    nc.sync.dma_start(
                        out=wop[hp][:, :], in_=wo_d[hp * 128:(hp + 1) * 128, :]
                    )

                def w_t_slice(name, di, lo, hi):
                    return w_sb[name][:, di * HL + lo:di * HL + hi]

                # Q^T, K^T: [head-pair 128, S]. di outer / qt inner so the
                # stationary weight chunk is reused across 4 matmuls.
                # PSUM drains split vector/scalar (ACT is idle in phase 1).
                # Group order hc-major: after (wk, hc0) both hp0 tensors
                # exist, so the first chunks' hp0 scores start right away.
                # Early-score drip: one score pair emitted between the
                # projection's di-groups, so the exps stream on the idle
                # ACT engine without the sc-ring (bufs=2) ever throttling
                # the PE to ACT pace.
                early_q = []

                def drip():
                    if early_q:
                        emit_scores_g(*early_q.pop(0))

                ndr = 0
                for wname, hc in (("wq", 0), ("wk", 0), ("wq", 1), ("wk", 1)):
                    dst = QT if wname == "wq" else KT
                    pss = [
                        proj_ps.tile([128, 512], F32, tag="pp", name="pp")
                        for _ in range(4)
                    ]
                    for di in range(8):
                        for qt in range(4):
                            nc.tensor.matmul(
                                pss[qt][:, :],
                                w_t_slice(wname, di, hc * 128, (hc + 1) * 128),
                                xT_t[di][:, qt * 512:(qt + 1) * 512],
                                start=(di == 0),
                                stop=(di == 7),
                            )
                        if di % 2 == 1:
                            drip()
                    for qt in range(4):
                        eng = nc.vector if ndr % 2 == 0 else nc.scalar
                        if eng is nc.scalar:
                            eng.copy(
                                dst[hc][:, qt * 512:(qt + 1) * 512], pss[qt][:, :]
                            )
                        else:
                            eng.tensor_copy(
                                dst[hc][:, qt * 512:(qt + 1) * 512], pss[qt][:, :]
                            )
                        ndr += 1
                    if (wname, hc) == ("wk", 0):
                        # qc0+qc1 hp0 scores: their exps run under the
                        # remaining QK projection matmuls
                        for qc_e in (0, 1):
                            for pt in range(qc_e * 4 + 4):
                                early_q.append((qc_e, pt, 0))
                    if (wname, hc) == ("wk", 1):
                        # qc1 hp1 scores: exps run under the V projection
                        for pt in range(8):
                            early_q.append((1, pt, 1))

                # V: [p, h] per p-chunk; one strided drain per chunk writes
                # around the pre-set ones columns.
                for pc in range(16):
                    ps = proj_ps.tile([128, 512], F32, tag="pp", name="pp")
                    for di in range(8):
                        nc.tensor.matmul(
                            ps[:, :HL],
                            xT_t[di][:, pc * 128:(pc + 1) * 128],
                            w_t_slice("wv", di, 0, HL),
                            start=(di == 0),
                            stop=(di == 7),
                        )
                    base = pc * (HPC * 65)
                    vdst = V_sb[:, base:base + HPC * 65].rearrange(
                        "p (h c) -> p h c", c=65
                    )[:, :, 0:64]
                    vsrc = ps[:, :HL].rearrange("p (h c) -> p h c", c=64)
                    # vector only: the ACT queue is full of early exps and
                    # the first AVs need V as soon as the projection ends
                    nc.vector.tensor_copy(vdst, vsrc)
                    drip()
                while early_q:
                    drip()

            # ---- Phase 2: attention (normalization deferred 1 q-chunk) ----
            # zu: unnormalized z^T per head [64, S]; lall row h = denominator
            # of head h; rqb = bf16(1/l). zup: normalized head-pair tiles
            # [128, S] feeding the K=128 O-projection.
            zu = [att_sb.tile([64, S], BF16, tag=f"zu{h}", name=f"zu{h}")
                  for h in range(HPC)]
            zup = [att_sb.tile([128, S], BF16, tag=f"zup{hp}", name=f"zup{hp}")
                   for hp in range(2)]
            # head h's denominator / reciprocal parked at partition 32h
            lall = att_sb.tile([128, S], F32, tag="lall", name="lall")
            rall = att_sb.tile([128, S], F32, tag="rall", name="rall")
            rqb = att_sb.tile([128, S], BF16, tag="rqb", name="rqb")
            nc.vector.memset(lall[:, :], 1.0)
            with tc.tile_pool(name="z_ps", bufs=4, space="PSUM") as z_ps:
                pending_norm = None
                pending_oproj = []

                def make_oproj(q0, qs):
                    # one output tile [128 q, 1024]: 4 K=128 pair matmuls in
                    # a recycled sc-ring tile, engine drain to bf16, DMA out
                    def _oproj(drain_eng=None):
                        pso = sc_ps.tile([128, 1024], F32, tag="sc", name="sc")
                        for dm in range(2):
                            for hp in range(2):
                                nc.tensor.matmul(
                                    pso[:, dm * 512:(dm + 1) * 512],
                                    zup[hp][:, q0 + qs * 128:q0 + (qs + 1) * 128],
                                    wop[hp][:, dm * 512:(dm + 1) * 512],
                                    start=(hp == 0),
                                    stop=(hp == 1),
                                )
                        ot = att_sb.tile([128, D], BF16, tag="ot", name="ot", bufs=4)
                        if drain_eng is nc.scalar:
                            nc.scalar.copy(ot[:, :], pso[:, :])
                        else:
                            nc.vector.tensor_copy(ot[:, :], pso[:, :])
                        nc.sync.dma_start(
                            out=out_d[q0 + qs * 128:q0 + (qs + 1) * 128, :],
                            in_=ot[:, :],
                        )
                    return _oproj

                def make_norm(q0):
                    # rb broadcast into a recycled sc-ring tile (both pairs
                    # side by side; f32 matmul straight from the f32
                    # reciprocal) + normalized pair tiles on vector. Using
                    # the sc ring instead of the z ring means this can be
                    # emitted late enough that the PE never waits on the
                    # DVE reciprocal chain.
                    def _norm():
                        rbt = sc_ps.tile([128, 1024], F32, tag="sc", name="sc")
                        for hp in range(2):
                            nc.tensor.matmul(
                                rbt[:, hp * 512:(hp + 1) * 512],
                                ind[hp][:, :], rqb[:, q0:q0 + QC],
                                start=True, stop=True,
                            )
                        for hp in range(2):
                            for i in range(2):
                                nc.vector.tensor_mul(
                                    zup[hp][64 * i:64 * i + 64, q0:q0 + QC],
                                    zu[2 * hp + i][:, q0:q0 + QC],
                                    rbt[64 * i:64 * i + 64, hp * 512:(hp + 1) * 512],
                                )
                    return _norm

                for qc in range(NQC):
                    q0 = qc * QC
                    npt = q0 // 128 + 4
                    zt = []  # allocated lazily at the first AV step
                    Ps = {}

                    def emit_scores(pt, hp):
                        if (qc, pt, hp) in PS_CACHE:
                            Ps[(pt, hp)] = PS_CACHE.pop((qc, pt, hp))
                            return
                        emit_scores_g(qc, pt, hp)
                        Ps[(pt, hp)] = PS_CACHE.pop((qc, pt, hp))

                    def emit_av(apt, hp):
                        ac0 = max(0, apt - q0 // 128) * 128
                        Pp = Ps.pop((apt, hp))
                        if not zt:
                            zt.extend(
                                z_ps.tile([128, 512], F32, tag="z", name="z")
                                for _ in range(HPC)
                            )
                        for i in range(2):
                            h = 2 * hp + i
                            src = (Pp[:, ac0:512] if i == 0
                                   else Pp[:, 512:1024 - ac0])
                            nc.tensor.matmul(
                                zt[h][0:65, ac0:],
                                V_sb[:, apt * (HPC * 65) + h * 65: apt * (HPC * 65) + (h + 1) * 65],
                                src,
                                start=(apt == 0),
                                stop=(apt == npt - 1),
                            )

                    # pair-granular software pipeline, 3-pair lag so the exp
                    # latency stays off the PE critical path. The previous
                    # chunk's deferred normalization slots in at step 6 (the
                    # reciprocal chain is done by then - no PE stall). While
                    # the first two chunks run on pre-computed P tiles, the
                    # third chunk's hp0 scores drip in to keep ACT fed.
                    steps = [(pt, hp) for pt in range(npt) for hp in range(2)]
                    LAG = 4
                    n_iter = len(steps) + LAG
                    sched = {}
                    if pending_norm is not None:
                        sched[min(7, n_iter - 1)] = [pending_norm]
                        pending_norm = None
                    if qc == 0:
                        for k, pt in enumerate(range(0, 4)):
                            sched.setdefault(3 + 2 * k, []).append(
                                (lambda pt=pt: emit_scores_g(2, pt, 0))
                            )
                    elif qc == 1:
                        for k, pt in enumerate(range(4, 12)):
                            sched.setdefault(3 + 2 * k, []).append(
                                (lambda pt=pt: emit_scores_g(2, pt, 0))
                            )
                    elif qc == 3:
                        # qc3 is exp-bound with PE and DVE slack: absorb the
                        # earlier chunks' O-projection tiles here (deps long
                        # settled; qc2's zup lands by ~step 9)
                        for k, fn in enumerate(pending_oproj):
                            sched.setdefault(min(8 + 2 * k, n_iter - 1), []).append(fn)
                        pending_oproj = []
                    # Splitting drains onto scalar only pays on the LAST
                    # chunk (ACT idle after its final exp); mid-kernel the
                    # scalar queue's exp backlog would delay z-bank
                    # recycling and the reciprocal chain by ~4us.
                    last_qc = qc == NQC - 1

                    def drain01():
                        eng0 = nc.scalar if last_qc else nc.vector
                        if eng0 is nc.scalar:
                            eng0.copy(
                                lall[0:1, q0:q0 + QC], zt[0][64:65, 0:512]
                            )
                            eng0.copy(zu[0][:, q0:q0 + QC], zt[0][0:64, 0:512])
                        else:
                            eng0.tensor_copy(
                                lall[0:1, q0:q0 + QC], zt[0][64:65, 0:512]
                            )
                            eng0.tensor_copy(
                                zu[0][:, q0:q0 + QC], zt[0][0:64, 0:512]
                            )
                        nc.vector.tensor_copy(
                            lall[32:33, q0:q0 + QC], zt[1][64:65, 0:512]
                        )
                        nc.vector.tensor_copy(
                            zu[1][:, q0:q0 + QC], zt[1][0:64, 0:512]
                        )

                    for n in range(n_iter):
                        if n < len(steps):
                            emit_scores(*steps[n])
                        for fn in sched.get(n, ()):
                            fn()
                        if n >= LAG:
                            emit_av(*steps[n - LAG])
                            # heads 0/1 stop one step before 2/3: start the
                            # reciprocal chain + their z drains early
                            if steps[n - LAG] == (npt - 1, 0):
                                drain01()

                    # ---- q-chunk tail: finish l gather, reciprocal, drain z
                    if last_qc:
                        nc.scalar.copy(
                            lall[64:65, q0:q0 + QC], zt[2][64:65, 0:512]
                        )
                    else:
                        nc.vector.tensor_copy(
                            lall[64:65, q0:q0 + QC], zt[2][64:65, 0:512]
                        )
                    nc.vector.tensor_copy(
                        lall[96:97, q0:q0 + QC], zt[3][64:65, 0:512]
                    )
                    nc.vector.reciprocal(rall[:, q0:q0 + QC], lall[:, q0:q0 + QC])
                    nc.gpsimd.tensor_copy(rqb[:, q0:q0 + QC], rall[:, q0:q0 + QC])
                    if last_qc:
                        nc.scalar.copy(zu[2][:, q0:q0 + QC], zt[2][0:64, 0:512])
                    else:
                        nc.vector.tensor_copy(
                            zu[2][:, q0:q0 + QC], zt[2][0:64, 0:512]
                        )
                    nc.vector.tensor_copy(zu[3][:, q0:q0 + QC], zt[3][0:64, 0:512])

                    pending_norm = make_norm(q0)
                    pending_oproj = pending_oproj + [
                        make_oproj(q0, qs) for qs in range(4)
                    ]

                # tail: last q-chunk's normalization + its 4 output tiles
                # (all earlier chunks' output tiles were absorbed into the
                # qc3 attention stream; ACT is idle here so half the drains
                # go to scalar)
                pending_norm()
                pending_norm = None
                for k, fn in enumerate(pending_oproj):
                    fn(drain_eng=nc.scalar if k % 2 == 0 else nc.vector)
                pending_oproj = []

            sc_pool_cm.__exit__(None, None, None)

    _split_multiwait(nc)
    return nc


def _prep_in_maps(x, W_K, W_Q, W_V, W_O):
    x = np.asarray(x, dtype=np.float32)
    W_K = np.asarray(W_K, dtype=np.float32)
    W_Q = np.asarray(W_Q, dtype=np.float32)
    W_V = np.asarray(W_V, dtype=np.float32)
    W_O = np.asarray(W_O, dtype=np.float32)

    import ml_dtypes
    bf16 = ml_dtypes.bfloat16
    pp, qq = np.meshgrid(np.arange(128), np.arange(128), indexing="ij")
    mask = np.where(qq >= pp, 1.0, 0.0).astype(bf16)

    in_maps = []
    for c in range(N_CORES):
        b, g = c // 4, c % 4
        hs = slice(HPC * g, HPC * g + HPC)
        xT = np.ascontiguousarray(x[b].T).astype(bf16)
        wq = np.ascontiguousarray(W_Q[hs].transpose(2, 0, 1).reshape(D, HL)).astype(bf16)
        wk = np.ascontiguousarray(W_K[hs].transpose(2, 0, 1).reshape(D, HL)).astype(bf16)
        wv = np.ascontiguousarray(W_V[hs].transpose(2, 0, 1).reshape(D, HL)).astype(bf16)
        wo = np.ascontiguousarray(W_O[:, HL * g:HL * g + HL].T).astype(bf16)
        in_maps.append(
            {"xT": xT, "wq": wq, "wk": wk, "wv": wv, "wo": wo, "mask": mask}
        )
    return in_maps


_NC_CACHE = None


def _get_nc():
    global _NC_CACHE
    if _NC_CACHE is None:
        _NC_CACHE = build_nc()
    return _NC_CACHE


def _run(x, W_K, W_Q, W_V, W_O, trace=False):
    nc = _get_nc()
    in_maps = _prep_in_maps(x, W_K, W_Q, W_V, W_O)
    res = run_bass_kernel_spmd(
        nc, in_maps, core_ids=list(range(N_CORES)), trace=trace
    )
    partials = np.stack(
        [np.asarray(res.results[c]["out"]).astype(np.float32) for c in range(N_CORES)]
    )
    out = np.empty((B, S, D), dtype=np.float32)
    out[0] = partials[0:4].sum(axis=0)
    out[1] = partials[4:8].sum(axis=0)
    return out, res


def kernel(x, W_K, W_Q, W_V, W_O):
    out, _ = _run(x, W_K, W_Q, W_V, W_O, trace=False)
    return out


def run_traced(x, W_K, W_Q, W_V, W_O):
    """For test.py: returns (out, BassKernelResults with exec_time_ns)."""
    import types

    if "antenv.axon_hooks" not in sys.modules:
        try:
            from trn_agent_boot.trn_boot import _ntff_profile_via_ctypes

            hook = _ntff_profile_via_ctypes("/opt/axon/libaxon_pjrt.so")
            mod = types.ModuleType("antenv.axon_hooks")
            mod.get_axon_ntff_profile_hook = lambda: hook
            mod.set_axon_ntff_profile_hook = lambda h: None
            sys.modules["antenv.axon_hooks"] = mod
        except Exception:
            pass
    return _run(x, W_K, W_Q, W_V, W_O, trace=True)


# revision 76
# speedup vs baseline: 1.0337x; 1.0139x over previous
"""Distributed causal multi-head attention for 8 TRN2 NeuronCores.

Problem: x[2, 2048, 1024], 16 heads x 64 dim, causal softmax attention,
output projection. Sharding: tensor-parallel over (batch, head-group):
core c handles batch c//4 and heads [4*(c%4), 4*(c%4)+4). Each core
computes its 4 heads' attention plus the partial output projection
(sum over its heads); the host sums the 4 partials per batch.

On-device layout strategy (no transposes anywhere on device):
  - host feeds xT = x[b].T               [D=1024, S=2048]
  - wq/wk/wv = W[heads] as [D, 256]      (d-major, head-major columns)
  - wo pair  = W_O rows per head pair    [128, 1024]
  - Q^T/K^T computed as [head-pair 128, S]; V as [p, 65*4] with a ones
    column folded per head so the attention-value matmul also produces
    the softmax denominator row.
  - scores tile per (p-tile, head pair) is one 2-bank PSUM tile laid
    [h0 | h1] with the causally-dead c0 columns squeezed out so a single
    contiguous ACT exp covers both heads with no garbage columns (ACT
    paces the attention pipeline; every element counts).
  - causal handling: fully-masked 128-col blocks are skipped in
    scores/exp/AV; the true-diagonal 128x128 block gets a multiplicative
    tril on the probabilities after exp (keeps DVE off the ACT path).
  - z^T accumulated in PSUM [65, 512] per head (row 64 = denominator l).
  - normalization (deferred one q-chunk, emitted at step 7 of the next
    chunk so the PE never waits on it): l rows gathered to partitions
    32h, one [128,512] reciprocal on DVE, bf16 cast on gpsimd, then a
    one-hot [128,128] indicator matmul per head pair broadcasts r across
    partitions into a recycled sc-ring PSUM tile; DVE muls produce the
    normalized pair tile zup[hp] [128, S] consumed by the O-projection.
  - O-projection: out[q,1024] accumulates TWO K=128 matmuls per output
    tile (head pairs stacked on the contraction axis) - full PE array,
    half the instruction count of per-head K=64 matmuls.

Matmul compute dtype: bfloat16 (full-rate on TRN2; rel err ~6e-3 vs the
fp32 reference), fp32 accumulation in PSUM. The per-q 1/l factors are
bf16 (adds <0.4% rms; the 2e-2 gate has plenty of margin).

Schedule notes:
  - Score matmuls for the two heads of a pair sit at PE row groups 0/64
    (tile_position) and execute CONCURRENTLY on the array - confirmed
    from the perfetto trace (pairs overlap ~90%).
  - AV matmuls run LAG=4 pair-steps behind scores to hide exp latency.
  - The attention phase's exp work (~90us at ~115G elem/s on ACT) is
    spread into every ACT-idle window: the first two q-chunks' scores
    are dripped one pair per projection di-group (exps run under the
    projection matmuls), the third chunk's hp0 scores drip through the
    first two chunks' (AV-dominated) attention, and the earlier chunks'
    O-projection tiles are absorbed into the exp-bound fourth chunk's
    stream via recycled sc-ring PSUM tiles. Held P tiles wait in a deep
    ring (bufs=36) until their AV matmuls consume them; only the last
    chunk's reciprocal chain + 4 output tiles remain as the tail.
"""

import sys

if "/opt/trn_rl_repo" not in sys.path:
    sys.path.insert(0, "/opt/trn_rl_repo")

import numpy as np

import concourse.bass as bass
import concourse.mybir as mybir
import concourse.tile as tile
from concourse.bass_utils import run_bass_kernel_spmd

B = 2
S = 2048
D = 1024
NH = 16
DH = 64
N_CORES = 8
HPC = 4          # heads per core
HL = HPC * DH    # 256 local head dims
QC = 512         # q-chunk width
NQC = S // QC

F32 = mybir.dt.float32
BF16 = mybir.dt.bfloat16
EXP = mybir.ActivationFunctionType.Exp


def _split_multiwait(nc, max_waits=1):
    """Walrus (CoreV3) rejects instructions carrying more than one sync
    wait; split extras into single-wait nops inserted before, same engine."""
    for f in nc.m.functions:
        for blk in f.blocks:
            insts = blk.instructions
            idx = 0
            while idx < len(insts):
                inst = insts[idx]
                si = getattr(inst, "sync_info", None)
                waits = list(si.on_wait) if si is not None else []
                if len(waits) > max_waits:
                    extra, keep = waits[:-max_waits], waits[-max_waits:]
                    si.on_wait = keep
                    for j, w in enumerate(extra):
                        nop = mybir.InstNoOp(
                            name=f"{inst.name}_sw{j}",
                            engine=inst.engine,
                            sync_info=mybir.SyncInfo(on_wait=[w], on_update=[]),
                            bass_nofuse=True,
                        )
                        insts.insert(idx, nop)
                        idx += 1
                idx += 1


def build_nc():
    nc = bass.Bass("TRN2", target_bir_lowering=False, debug=False, num_devices=N_CORES)

    xT_d = nc.declare_dram_parameter("xT", [D, S], BF16, isOutput=False)
    wq_d = nc.declare_dram_parameter("wq", [D, HL], BF16, isOutput=False)
    wk_d = nc.declare_dram_parameter("wk", [D, HL], BF16, isOutput=False)
    wv_d = nc.declare_dram_parameter("wv", [D, HL], BF16, isOutput=False)
    wo_d = nc.declare_dram_parameter("wo", [HL, D], BF16, isOutput=False)
    mask_d = nc.declare_dram_parameter("mask", [128, 128], BF16, isOutput=False)
    out_d = nc.declare_dram_parameter("out", [S, D], BF16, isOutput=True)

    with tile.TileContext(nc) as tc:
        with (
            tc.tile_pool(name="live_sb", bufs=1) as live_sb,
            tc.tile_pool(name="att_sb", bufs=1) as att_sb,
        ):
            # Tensors that live through the whole kernel.
            QT = [live_sb.tile([128, S], BF16, tag=f"QT{hc}", name=f"QT{hc}") for hc in range(2)]
            KT = [live_sb.tile([128, S], BF16, tag=f"KT{hc}", name=f"KT{hc}") for hc in range(2)]
            # V with a ones column per head: 16 p-chunks x [V0|1|V1|1|V2|1|V3|1]
            V_sb = live_sb.tile([128, 16 * (HPC * 65)], BF16, tag="V", name="V")
            wop = [live_sb.tile([128, D], BF16, tag=f"wop{hp}", name=f"wop{hp}") for hp in range(2)]
            mask_t = live_sb.tile([128, 128], BF16, tag="mask", name="mask")

            # Indicator weights for the r broadcast matmul (bf16: full PE
            # rate; the f32->bf16 cast of 1/l runs on the idle gpsimd well
            # off the critical path). Head h's 1/l row lives at partition
            # 32h (engine ops need 32-aligned bases). ind_hp is a one-hot
            # [128, 128] selecting partition 64hp -> output rows 0-63 and
            # 64hp+32 -> rows 64-127.
            indf = live_sb.tile([128, 128], F32, tag="indf", name="indf")
            ind = [live_sb.tile([128, 128], BF16, tag=f"ind{hp}", name=f"ind{hp}")
                   for hp in range(2)]
            for hp in range(2):
                nc.vector.memset(indf[:, :], 0.0)
                nc.vector.memset(indf[64 * hp:64 * hp + 1, 0:64], 1.0)
                nc.vector.memset(indf[64 * hp + 32:64 * hp + 33, 64:128], 1.0)
                nc.vector.tensor_copy(ind[hp][:, :], indf[:, :])

            # ones columns of V (col 64 of each head block), one strided memset
            ones_ap = V_sb[:, :].rearrange("p (a c) -> p a c", c=65)[:, :, 64:65]
            nc.vector.memset(ones_ap, 1.0)

            # The score-tile PSUM ring is open through phases 1+2: the
            # first two q-chunks' hp0 scores (and qc1's hp1) are emitted
            # during the projection phase so their exps run on the
            # otherwise-idle ACT engine under the projection matmuls.
            sc_pool_cm = tc.tile_pool(name="sc_ps", bufs=2, space="PSUM")
            sc_ps = sc_pool_cm.__enter__()
            PS_CACHE = {}

            def emit_scores_g(qc_, pt, hp):
                q0_ = qc_ * QC
                jj = pt - q0_ // 128  # >=0 means diagonal region
                c0 = max(0, jj) * 128
                # pair layout [h0 | h1] with dead columns squeezed:
                # h0 -> cols [c0:512], h1 -> cols [512:1024-c0], so one
                # contiguous exp covers both heads, no garbage.
                scp = sc_ps.tile([128, 1024], F32, tag="sc", name="sc")
                for i in range(2):
                    h = 2 * hp + i
                    hc2, ho = h // 2, (h % 2) * 64
                    dst = (scp[:, c0:512] if i == 0 else scp[:, 512:1024 - c0])
                    nc.tensor.matmul(
                        dst,
                        KT[hc2][ho:ho + 64, pt * 128:pt * 128 + 128],
                        QT[hc2][ho:ho + 64, q0_ + c0:q0_ + QC],
                        start=True,
                        stop=True,
                        tile_position=(ho, 0),
                    )
                Pp = att_sb.tile([128, 1024], BF16, tag="P", name="P", bufs=36)
                nc.scalar.activation(
                    Pp[:, c0:1024 - c0], scp[:, c0:1024 - c0], EXP, scale=0.125
                )
                if jj >= 0:
                    # causal tril applied multiplicatively post-exp; all-SBUF
                    # bf16 work on the otherwise idle gpsimd (DVE saturates
                    # at chunk boundaries; LAG gives gpsimd plenty of slack)
                    blk0 = slice(jj * 128, (jj + 1) * 128)
                    nc.gpsimd.tensor_mul(Pp[:, blk0], Pp[:, blk0], mask_t[:, :])
                    nc.gpsimd.tensor_mul(Pp[:, 512:640], Pp[:, 512:640], mask_t[:, :])
                PS_CACHE[(qc_, pt, hp)] = Pp

            # ---- Phase 1: projections (xT and w tiles scoped here) ----
            with (
                tc.tile_pool(name="xw_sb", bufs=1) as xw_sb,
                tc.tile_pool(name="proj_ps", bufs=4, space="PSUM") as proj_ps,
            ):
                # DMA issue order matters: the projection's first matmul
                # needs wq + xT chunk 0, so those go first; wk before the
                # x tail so the K^T loop never waits on it.
                w_sb = {}
                w_tiles = {}
                for name in ("wq", "wk", "wv"):
                    w_tiles[name] = xw_sb.tile(
                        [128, 8 * HL], BF16, tag=f"{name}b", name=f"{name}b"
                    )

                def _w_dma(name, dram):
                    t = w_tiles[name]
                    src = dram.ap().rearrange("(di p) h -> di p h", p=128).transpose((1, 0, 2))
                    dst = t[:, :].rearrange("p (di h) -> p di h", di=8)
                    nc.sync.dma_start(out=dst, in_=src)
                    w_sb[name] = t

                xT_t = [
                    xw_sb.tile([128, S], BF16, tag=f"x{di}", name=f"x{di}")
                    for di in range(8)
                ]

                def _x_dma(di, eng=nc.sync):
                    eng.dma_start(
                        out=xT_t[di][:, :], in_=xT_d[di * 128:(di + 1) * 128, :]
                    )

                _w_dma("wq", wq_d)
                _x_dma(0)
                _x_dma(1)
                _w_dma("wk", wk_d)
                for di in range(2, 8):
                    _x_dma(di)
                _w_dma("wv", wv_d)
                nc.sync.dma_start(out=mask_t[:, :], in_=mask_d[:, :])
                for hp in range(2):
                    nc.sync.dma_start(
                        out=wop[hp][:, :], in_=wo_d[hp * 128:(hp + 1) * 128, :]
                    )

                def w_t_slice(name, di, lo, hi):
                    return w_sb[name][:, di * HL + lo:di * HL + hi]

                # Q^T, K^T: [head-pair 128, S]. di outer / qt inner so the
                # stationary weight chunk is reused across 4 matmuls.
                # PSUM drains split vector/scalar (ACT is idle in phase 1).
                # Group order hc-major: after (wk, hc0) both hp0 tensors
                # exist, so the first chunks' hp0 scores start right away.
                # Early-score drip: one score pair emitted between the
                # projection's di-groups, so the exps stream on the idle
                # ACT engine without the sc-ring (bufs=2) ever throttling
                # the PE to ACT pace.
                early_q = []

                def drip():
                    if early_q:
                        emit_scores_g(*early_q.pop(0))

                ndr = 0
                for wname, hc in (("wq", 0), ("wk", 0), ("wq", 1), ("wk", 1)):
                    dst = QT if wname == "wq" else KT
                    pss = [
                        proj_ps.tile([128, 512], F32, tag="pp", name="pp")
                        for _ in range(4)
                    ]
                    for di in range(8):
                        for qt in range(4):
                            nc.tensor.matmul(
                                pss[qt][:, :],
                                w_t_slice(wname, di, hc * 128, (hc + 1) * 128),
                                xT_t[di][:, qt * 512:(qt + 1) * 512],
                                start=(di == 0),
                                stop=(di == 7),
                            )
                        if di % 2 == 1:
                            drip()
                    for qt in range(4):
                        eng = nc.vector if ndr % 2 == 0 else nc.scalar
                        if eng is nc.scalar:
                            eng.copy(
                                dst[hc][:, qt * 512:(qt + 1) * 512], pss[qt][:, :]
                            )
                        else:
                            eng.tensor_copy(
                                dst[hc][:, qt * 512:(qt + 1) * 512], pss[qt][:, :]
                            )
                        ndr += 1
                    if (wname, hc) == ("wk", 0):
                        # qc0+qc1 hp0 scores: their exps run under the
                        # remaining QK projection matmuls
                        for qc_e in (0, 1):
                            for pt in range(qc_e * 4 + 4):
                                early_q.append((qc_e, pt, 0))
                    if (wname, hc) == ("wk", 1):
                        # qc1 hp1 scores: exps run under the V projection
                        for pt in range(8):
                            early_q.append((1, pt, 1))

                # V: [p, h] per p-chunk; one strided drain per chunk writes
                # around the pre-set ones columns.
                for pc in range(16):
                    ps = proj_ps.tile([128, 512], F32, tag="pp", name="pp")
                    for di in range(8):
                        nc.tensor.matmul(
                            ps[:, :HL],
                            xT_t[di][:, pc * 128:(pc + 1) * 128],
                            w_t_slice("wv", di, 0, HL),
                            start=(di == 0),
                            stop=(di == 7),
                        )
                    base = pc * (HPC * 65)
                    vdst = V_sb[:, base:base + HPC * 65].rearrange(
                        "p (h c) -> p h c", c=65
                    )[:, :, 0:64]
                    vsrc = ps[:, :HL].rearrange("p (h c) -> p h c", c=64)
                    # vector only: the ACT queue is full of early exps and
                    # the first AVs need V as soon as the projection ends
                    nc.vector.tensor_copy(vdst, vsrc)
                    drip()
                while early_q:
                    drip()

            # ---- Phase 2: attention (normalization deferred 1 q-chunk) ----
            # zu: unnormalized z^T per head [64, S]; lall row h = denominator
            # of head h; rqb = bf16(1/l). zup: normalized head-pair tiles
            # [128, S] feeding the K=128 O-projection.
            zu = [att_sb.tile([64, S], BF16, tag=f"zu{h}", name=f"zu{h}")
                  for h in range(HPC)]
            zup = [att_sb.tile([128, S], BF16, tag=f"zup{hp}", name=f"zup{hp}")
                   for hp in range(2)]
            # head h's denominator / reciprocal parked at partition 32h
            lall = att_sb.tile([128, S], F32, tag="lall", name="lall")
            rall = att_sb.tile([128, S], F32, tag="rall", name="rall")
            rqb = att_sb.tile([128, S], BF16, tag="rqb", name="rqb")
            nc.vector.memset(lall[:, :], 1.0)
            with tc.tile_pool(name="z_ps", bufs=4, space="PSUM") as z_ps:
                pending_norm = None
                pending_oproj = []

                def make_oproj(q0, qs):
                    # one output tile [128 q, 1024]: 4 K=128 pair matmuls in
                    # a recycled sc-ring tile, engine drain to bf16, DMA out
                    def _oproj(drain_eng=None):
                        pso = sc_ps.tile([128, 1024], F32, tag="sc", name="sc")
                        for dm in range(2):
                            for hp in range(2):
                                nc.tensor.matmul(
                                    pso[:, dm * 512:(dm + 1) * 512],
                                    zup[hp][:, q0 + qs * 128:q0 + (qs + 1) * 128],
                                    wop[hp][:, dm * 512:(dm + 1) * 512],
                                    start=(hp == 0),
                                    stop=(hp == 1),
                                )
                        ot = att_sb.tile([128, D], BF16, tag="ot", name="ot", bufs=4)
                        if drain_eng is nc.scalar:
                            nc.scalar.copy(ot[:, :], pso[:, :])
                        else:
                            nc.vector.tensor_copy(ot[:, :], pso[:, :])
                        nc.sync.dma_start(
                            out=out_d[q0 + qs * 128:q0 + (qs + 1) * 128, :],
                            in_=ot[:, :],
                        )
                    return _oproj

                def make_norm(q0):
                    # rb broadcast into a recycled sc-ring tile (both pairs
                    # side by side; f32 matmul straight from the f32
                    # reciprocal) + normalized pair tiles on vector. Using
                    # the sc ring instead of the z ring means this can be
                    # emitted late enough that the PE never waits on the
                    # DVE reciprocal chain.
                    def _norm():
                        rbt = sc_ps.tile([128, 1024], F32, tag="sc", name="sc")
                        for hp in range(2):
                            nc.tensor.matmul(
                                rbt[:, hp * 512:(hp + 1) * 512],
                                ind[hp][:, :], rqb[:, q0:q0 + QC],
                                start=True, stop=True,
                            )
                        for hp in range(2):
                            for i in range(2):
                                nc.vector.tensor_mul(
                                    zup[hp][64 * i:64 * i + 64, q0:q0 + QC],
                                    zu[2 * hp + i][:, q0:q0 + QC],
                                    rbt[64 * i:64 * i + 64, hp * 512:(hp + 1) * 512],
                                )
                    return _norm

                for qc in range(NQC):
                    q0 = qc * QC
                    npt = q0 // 128 + 4
                    zt = []  # allocated lazily at the first AV step
                    Ps = {}

                    def emit_scores(pt, hp):
                        if (qc, pt, hp) in PS_CACHE:
                            Ps[(pt, hp)] = PS_CACHE.pop((qc, pt, hp))
                            return
                        emit_scores_g(qc, pt, hp)
                        Ps[(pt, hp)] = PS_CACHE.pop((qc, pt, hp))

                    def emit_av(apt, hp):
                        ac0 = max(0, apt - q0 // 128) * 128
                        Pp = Ps.pop((apt, hp))
                        if not zt:
                            zt.extend(
                                z_ps.tile([128, 512], F32, tag="z", name="z")
                                for _ in range(HPC)
                            )
                        for i in range(2):
                            h = 2 * hp + i
                            src = (Pp[:, ac0:512] if i == 0
                                   else Pp[:, 512:1024 - ac0])
                            nc.tensor.matmul(
                                zt[h][0:65, ac0:],
                                V_sb[:, apt * (HPC * 65) + h * 65: apt * (HPC * 65) + (h + 1) * 65],
                                src,
                                start=(apt == 0),
                                stop=(apt == npt - 1),
                            )

                    # pair-granular software pipeline, 3-pair lag so the exp
                    # latency stays off the PE critical path. The previous
                    # chunk's deferred normalization slots in at step 6 (the
                    # reciprocal chain is done by then - no PE stall). While
                    # the first two chunks run on pre-computed P tiles, the
                    # third chunk's hp0 scores drip in to keep ACT fed.
                    steps = [(pt, hp) for pt in range(npt) for hp in range(2)]
                    LAG = 4
                    n_iter = len(steps) + LAG
                    sched = {}
                    if pending_norm is not None:
                        # the boundary DVE chain (early drains + l gather +
                        # reciprocal + cast) takes ~9us; step 10 is the
                        # earliest the rb matmul never waits
                        sched[min(10, n_iter - 1)] = [pending_norm]
                        pending_norm = None
                    if qc == 0:
                        for k, pt in enumerate(range(0, 4)):
                            sched.setdefault(3 + 2 * k, []).append(
                                (lambda pt=pt: emit_scores_g(2, pt, 0))
                            )
                    elif qc == 1:
                        for k, pt in enumerate(range(4, 12)):
                            sched.setdefault(3 + 2 * k, []).append(
                                (lambda pt=pt: emit_scores_g(2, pt, 0))
                            )
                    elif qc == 3:
                        # qc3 is exp-bound with PE and DVE slack: absorb the
                        # earlier chunks' O-projection tiles here (deps long
                        # settled; qc2's zup lands by ~step 9)
                        for k, fn in enumerate(pending_oproj):
                            sched.setdefault(min(8 + 2 * k, n_iter - 1), []).append(fn)
                        pending_oproj = []
                    # Splitting drains onto scalar only pays on the LAST
                    # chunk (ACT idle after its final exp); mid-kernel the
                    # scalar queue's exp backlog would delay z-bank
                    # recycling and the reciprocal chain by ~4us.
                    last_qc = qc == NQC - 1

                    def drain01():
                        eng0 = nc.scalar if last_qc else nc.vector
                        if eng0 is nc.scalar:
                            eng0.copy(
                                lall[0:1, q0:q0 + QC], zt[0][64:65, 0:512]
                            )
                            eng0.copy(zu[0][:, q0:q0 + QC], zt[0][0:64, 0:512])
                        else:
                            eng0.tensor_copy(
                                lall[0:1, q0:q0 + QC], zt[0][64:65, 0:512]
                            )
                            eng0.tensor_copy(
                                zu[0][:, q0:q0 + QC], zt[0][0:64, 0:512]
                            )
                        nc.vector.tensor_copy(
                            lall[32:33, q0:q0 + QC], zt[1][64:65, 0:512]
                        )
                        nc.vector.tensor_copy(
                            zu[1][:, q0:q0 + QC], zt[1][0:64, 0:512]
                        )

                    for n in range(n_iter):
                        if n < len(steps):
                            emit_scores(*steps[n])
                        for fn in sched.get(n, ()):
                            fn()
                        if n >= LAG:
                            emit_av(*steps[n - LAG])
                            # heads 0/1 stop one step before 2/3: start the
                            # reciprocal chain + their z drains early
                            if steps[n - LAG] == (npt - 1, 0):
                                drain01()

                    # ---- q-chunk tail: finish l gather, drain z, reciprocal
                    # z drains go FIRST on DVE: the next chunk's heads-2/3
                    # AVs wait on these banks, while the reciprocal isn't
                    # consumed until step 10. At the last chunk, zu2/zu3 go
                    # to the idle scalar engine so the half-reciprocals jump
                    # straight to the head of the DVE queue.
                    if last_qc:
                        nc.scalar.copy(
                            lall[64:65, q0:q0 + QC], zt[2][64:65, 0:512]
                        )
                    else:
                        nc.vector.tensor_copy(
                            lall[64:65, q0:q0 + QC], zt[2][64:65, 0:512]
                        )
                    nc.vector.tensor_copy(
                        lall[96:97, q0:q0 + QC], zt[3][64:65, 0:512]
                    )
                    if last_qc:
                        nc.scalar.copy(zu[2][:, q0:q0 + QC], zt[2][0:64, 0:512])
                        nc.scalar.copy(zu[3][:, q0:q0 + QC], zt[3][0:64, 0:512])
                    else:
                        nc.vector.tensor_copy(
                            zu[2][:, q0:q0 + QC], zt[2][0:64, 0:512]
                        )
                        nc.vector.tensor_copy(
                            zu[3][:, q0:q0 + QC], zt[3][0:64, 0:512]
                        )
                        nc.vector.reciprocal(
                            rall[:, q0:q0 + QC], lall[:, q0:q0 + QC]
                        )
                        nc.gpsimd.tensor_copy(
                            rqb[:, q0:q0 + QC], rall[:, q0:q0 + QC]
                        )

                    pending_norm = make_norm(q0)
                    pending_oproj = pending_oproj + [
                        make_oproj(q0, qs) for qs in range(4)
                    ]

                # half-pipelined tail for the last chunk (earlier chunks'
                # output tiles were absorbed into the qc3 stream): the
                # reciprocal -> broadcast -> normalize -> O-proj chain runs
                # per 256-col half so the first output tiles start ~3us
                # sooner; ACT is idle here so half the drains go to scalar
                q0t = (NQC - 1) * QC
                for half in range(2):
                    c = q0t + half * 256
                    nc.vector.reciprocal(rall[:, c:c + 256], lall[:, c:c + 256])
                    nc.gpsimd.tensor_copy(rqb[:, c:c + 256], rall[:, c:c + 256])
                for half in range(2):
                    c = q0t + half * 256
                    rbt = sc_ps.tile([128, 512], F32, tag="sc", name="sc")
                    for hp in range(2):
                        nc.tensor.matmul(
                            rbt[:, hp * 256:(hp + 1) * 256], ind[hp][:, :],
                            rqb[:, c:c + 256], start=True, stop=True,
                        )
                    for hp in range(2):
                        for i in range(2):
                            nc.vector.tensor_mul(
                                zup[hp][64 * i:64 * i + 64, c:c + 256],
                                zu[2 * hp + i][:, c:c + 256],
                                rbt[64 * i:64 * i + 64, hp * 256:(hp + 1) * 256],
                            )
                    for k in range(2):
                        pending_oproj[half * 2 + k](
                            drain_eng=nc.scalar if k == 0 else nc.vector
                        )
                pending_norm = None
                pending_oproj = []

            sc_pool_cm.__exit__(None, None, None)

    _split_multiwait(nc)
    return nc


def _prep_in_maps(x, W_K, W_Q, W_V, W_O):
    x = np.asarray(x, dtype=np.float32)
    W_K = np.asarray(W_K, dtype=np.float32)
    W_Q = np.asarray(W_Q, dtype=np.float32)
    W_V = np.asarray(W_V, dtype=np.float32)
    W_O = np.asarray(W_O, dtype=np.float32)

    import ml_dtypes
    bf16 = ml_dtypes.bfloat16
    pp, qq = np.meshgrid(np.arange(128), np.arange(128), indexing="ij")
    mask = np.where(qq >= pp, 1.0, 0.0).astype(bf16)

    in_maps = []
    for c in range(N_CORES):
        b, g = c // 4, c % 4
        hs = slice(HPC * g, HPC * g + HPC)
        xT = np.ascontiguousarray(x[b].T).astype(bf16)
        wq = np.ascontiguousarray(W_Q[hs].transpose(2, 0, 1).reshape(D, HL)).astype(bf16)
        wk = np.ascontiguousarray(W_K[hs].transpose(2, 0, 1).reshape(D, HL)).astype(bf16)
        wv = np.ascontiguousarray(W_V[hs].transpose(2, 0, 1).reshape(D, HL)).astype(bf16)
        wo = np.ascontiguousarray(W_O[:, HL * g:HL * g + HL].T).astype(bf16)
        in_maps.append(
            {"xT": xT, "wq": wq, "wk": wk, "wv": wv, "wo": wo, "mask": mask}
        )
    return in_maps


_NC_CACHE = None


def _get_nc():
    global _NC_CACHE
    if _NC_CACHE is None:
        _NC_CACHE = build_nc()
    return _NC_CACHE


def _run(x, W_K, W_Q, W_V, W_O, trace=False):
    nc = _get_nc()
    in_maps = _prep_in_maps(x, W_K, W_Q, W_V, W_O)
    res = run_bass_kernel_spmd(
        nc, in_maps, core_ids=list(range(N_CORES)), trace=trace
    )
    partials = np.stack(
        [np.asarray(res.results[c]["out"]).astype(np.float32) for c in range(N_CORES)]
    )
    out = np.empty((B, S, D), dtype=np.float32)
    out[0] = partials[0:4].sum(axis=0)
    out[1] = partials[4:8].sum(axis=0)
    return out, res


def kernel(x, W_K, W_Q, W_V, W_O):
    out, _ = _run(x, W_K, W_Q, W_V, W_O, trace=False)
    return out


def run_traced(x, W_K, W_Q, W_V, W_O):
    """For test.py: returns (out, BassKernelResults with exec_time_ns)."""
    import types

    if "antenv.axon_hooks" not in sys.modules:
        try:
            from trn_agent_boot.trn_boot import _ntff_profile_via_ctypes

            hook = _ntff_profile_via_ctypes("/opt/axon/libaxon_pjrt.so")
            mod = types.ModuleType("antenv.axon_hooks")
            mod.get_axon_ntff_profile_hook = lambda: hook
            mod.set_axon_ntff_profile_hook = lambda h: None
            sys.modules["antenv.axon_hooks"] = mod
        except Exception:
            pass
    return _run(x, W_K, W_Q, W_V, W_O, trace=True)
